# revision 21
# baseline (speedup 1.0000x reference)
"""GPT-OSS expert MLP (gate/up GEMM + clamped GLU + down GEMM + routing scale)
on 8 Trainium2 NeuronCores.

Sharding: tensor-parallel split of the intermediate dim I=2880 across 8 cores
(360 columns each, padded to 384 = 3*128). Each core computes
  gate/up = hidden @ W[:, slice] ; glu ; y_partial = glu_h @ down_w[slice, :]
and writes its full [H, T] partial (transposed layout). The host sums the 8
partials, applies down bias, routing weights, and the residual add.

All matmul operands are bf16: the quantized weights (values k/32, |k|<=4) are
exactly representable in bf16, so the only rounding is on hidden_states.
PSUM accumulation is fp32; partials are written out in bf16 and
summed on the host in fp64.
"""

import numpy as np
import ml_dtypes

BF16 = ml_dtypes.bfloat16

H = 2880          # hidden size
I = 2880          # intermediate size
T = 512           # tokens (full problem size)
NCORES = 8
IC = I // NCORES  # 360 intermediate cols per core
ICP = 384         # padded to 3 * 128
MT = ICP // 128   # 3 i-tiles per core
HP = 2944         # H padded to 23 * 128
KT = HP // 128    # 23 k-tiles over hidden dim
ALPHA = 1.702
LIMIT = 7.0
_cache = {}


def routing_compaction(expert_mask, routing_weights):
    """Tokens with sum_j mask[j,t]*rw[t,j] == 0 contribute exactly
    final_hidden_states[t] to the output, so the device only computes the
    active tokens. Returns (active_idx, tok_w, tp) with tp = active count
    padded up to a multiple of 64 (the compiled moving-dim size)."""
    mask = np.asarray(expert_mask, np.float32)
    rw = np.asarray(routing_weights, np.float32)
    tok_w = np.einsum("jt,tj->t", mask, rw)
    active = np.flatnonzero(tok_w)
    tp = max(64, -(-len(active) // 64) * 64)
    return active, tok_w, tp


def build_program(loop_reps=None, unroll_bodies=None, unroll=8, tp=448,
                  probe=None):
    """Build (and compile) the per-core Bass program. Identical on all cores;
    per-core data comes from in_maps. If loop_reps is given, the body is
    wrapped in a hardware For_i loop with `unroll` bodies per iteration
    (timing only): consecutive bodies pipeline via normal Tile semaphores
    (body i+1's loads overlap body i's down phase), and the ~2us all-engine
    back-edge barrier amortizes over `unroll` bodies.
    unroll_bodies=N emits the body N times sequentially with no loop (sim)."""
    import concourse.bacc as bacc
    import concourse.mybir as mybir
    import concourse.tile as tile

    fp32 = mybir.dt.float32
    bf16 = mybir.dt.bfloat16

    nc = bacc.Bacc("TRN2", target_bir_lowering=False, debug=False,
                   num_devices=NCORES)

    TT = tp            # active-token count (moving dim)
    hid_d = nc.dram_tensor("hid", [128, KT * TT], bf16, kind="ExternalInput").ap()
    gu_d = nc.dram_tensor("gu", [128, 2 * MT * KT * 128], bf16,
                          kind="ExternalInput").ap()
    dw_d = nc.dram_tensor("dw", [128, KT * MT * 128], bf16,
                          kind="ExternalInput").ap()
    gb_d = nc.dram_tensor("gb", [128, MT], fp32, kind="ExternalInput").ap()
    ub_d = nc.dram_tensor("ub", [128, MT], fp32, kind="ExternalInput").ap()
    y_d = nc.dram_tensor("y", [HP, TT], bf16, kind="ExternalOutput").ap()

    shared_tiles = {}

    def body(ctx, tc, pools, staged=False, load_this_body=True):
        wpool, glupool, hglupool, psum, psum_y, ypool = pools
        do_loads = (probe in (None, 'loads')) or load_this_body
        do_compute = probe in (None, 'pe', 'pe1w', 'gu', 'down')
        do_gu = do_compute and (probe != 'down' or load_this_body)
        do_down = do_compute and probe != 'gu'
        do_stores = probe is None

        def wtile(shape, dtype, tag):
            # probe='pe': reuse one tile object per tag across bodies so
            # unloaded bodies don't read fresh never-written allocations
            if probe not in ('pe', 'pe1w', 'gu', 'down'):
                return wpool.tile(shape, dtype, tag=tag, name=f'w_{tag}')
            if tag not in shared_tiles:
                shared_tiles[tag] = wpool.tile(shape, dtype, tag=tag,
                                                name=f'sh_{tag}')
            return shared_tiles[tag]

        # ---- loads, interleaved so PE's first needs arrive first ----
        # SP HWDGE ring is FIFO; emit in PE consumption order. Early pieces
        # are small so the PE can start ~3us in and m0-gate paces with the
        # hidden-state stream; later pieces are bigger (less issue overhead).
        hid_t = [None] * KT                    # kt -> (tile, col offset)
        gu_piece_sizes = {0: [6, 6, 6, 5], 1: [6, 6, 6, 5],
                          2: [12, 11], 3: [12, 11],
                          4: [12, 11], 5: [12, 11]}
        hid_piece_sizes = [2, 2, 2, 2, 3, 3, 3, 3, 3]
        gu_kt = {g: 0 for g in range(6)}       # next kt per group
        hid_kt = [0]
        gu_map = {}                            # (grp, kt) -> (tile, j)

        def load_hid():
            ci = sum(1 for k in range(KT) if hid_t[k] is not None)
            nk = hid_piece_sizes.pop(0)
            kt0 = hid_kt[0]
            t = wtile([128, nk * TT], bf16, f"hid{ci}")
            if do_loads:
                nc.sync.dma_start(t[:], hid_d[:, kt0 * TT:(kt0 + nk) * TT])
            for j in range(nk):
                hid_t[kt0 + j] = (t, j)
            hid_kt[0] = kt0 + nk

        def load_gu_piece(grp, idx):
            nk = gu_piece_sizes[grp][idx]
            kt0 = gu_kt[grp]
            t = wtile([128, nk * 128], bf16, f"gu{grp}_{idx}")
            if do_loads:
                nc.sync.dma_start(
                    t[:], gu_d[:, grp * KT * 128 + kt0 * 128:
                               grp * KT * 128 + (kt0 + nk) * 128])
            for j in range(nk):
                gu_map[(grp, kt0 + j)] = (t, j)
            gu_kt[grp] = kt0 + nk

        def gu_lhsT(grp, kt):
            if probe == 'pe1w':
                grp, kt = 0, 0
            t, j = gu_map[(grp, kt)]
            return t[:, j * 128:(j + 1) * 128]

        load_gu_piece(0, 0); load_hid(); load_gu_piece(0, 1); load_hid()
        load_gu_piece(0, 2); load_hid(); load_gu_piece(0, 3); load_hid()
        gb_t = wtile([128, MT], fp32, "gb")
        ub_t = wtile([128, MT], fp32, "ub")
        if do_loads:
            nc.sync.dma_start(gb_t[:], gb_d[:])
            nc.sync.dma_start(ub_t[:], ub_d[:])
        load_gu_piece(1, 0); load_hid(); load_gu_piece(1, 1); load_hid()
        load_gu_piece(1, 2); load_hid(); load_gu_piece(1, 3); load_hid()
        load_hid()
        for grp in (2, 3, 4, 5):
            load_gu_piece(grp, 0); load_gu_piece(grp, 1)

        dw_t = wtile([128, KT * MT * 128], bf16, "dw")
        if do_loads:
            nc.sync.dma_start(dw_t[:], dw_d[:])

        # double-buffered: iteration i+1's GLU writes overlap iteration i's
        # down-matmul reads under the staggered-reset loop
        if probe == 'down':
            if 'hglu' not in shared_tiles:
                shared_tiles['hglu'] = hglupool.tile(
                    [128, MT * TT], bf16, tag="hglu", name="sh_hglu")
            hglu = shared_tiles['hglu']
        else:
            hglu = hglupool.tile([128, MT * TT], bf16, tag="hglu")

        def rhs(kt):
            t, j = hid_t[kt]
            return t[:, j * TT:(j + 1) * TT]

        # ---- gate/up GEMMs + GLU per i-tile ----
        if not do_compute:
            return
        for m in range(MT if do_gu else 0):
            if staged and m > 0:
                tc.stage_boundary()
            pg = psum.tile([128, TT], fp32, tag="pg")
            for kt in range(KT):
                nc.tensor.matmul(pg[:], gu_lhsT(2 * m, kt),
                                 rhs(kt), start=(kt == 0), stop=(kt == KT - 1))
            pu = psum.tile([128, TT], fp32, tag="pu")
            for kt in range(KT):
                nc.tensor.matmul(pu[:], gu_lhsT(2 * m + 1, kt),
                                 rhs(kt), start=(kt == 0), stop=(kt == KT - 1))

            # gate path: g = min(pg + gb, LIMIT); s = silu(ALPHA*g) = ALPHA*glu
            tg = glupool.tile([128, TT], fp32, tag="tg")
            nc.vector.tensor_scalar(tg[:], pg[:], gb_t[:, m:m + 1], LIMIT,
                                    mybir.AluOpType.add, mybir.AluOpType.min)
            sg = glupool.tile([128, TT], fp32, tag="sg")
            nc.scalar.activation(sg[:], tg[:],
                                 mybir.ActivationFunctionType.Silu, scale=ALPHA)
            # up path: u = clip(pu + ub, -LIMIT, LIMIT); u5 = u + 1
            tu = glupool.tile([128, TT], fp32, tag="tu")
            nc.vector.tensor_scalar(tu[:], pu[:], ub_t[:, m:m + 1], LIMIT,
                                    mybir.AluOpType.add, mybir.AluOpType.min)
            tu5 = glupool.tile([128, TT], fp32, tag="tu5")
            nc.vector.tensor_scalar(tu5[:], tu[:], -LIMIT, 1.0,
                                    mybir.AluOpType.max, mybir.AluOpType.add)
            # hglu = (ALPHA*glu) * (u+1); the 1/ALPHA is folded into dw
            nc.vector.tensor_tensor(hglu[:, m * TT:(m + 1) * TT], sg[:], tu5[:],
                                    mybir.AluOpType.mult)

        # ---- down GEMM, write bf16 partial y^TT ----
        # stores batched 4 h-tiles per DMA: per-store issue cost (~650ns
        # sequencer + ~625ns HWDGE) would otherwise pace the whole tail.
        # Stores go on the ACT HWDGE ring so they never block the SP ring,
        # which carries the next iteration's loads.
        if not do_down:
            return
        if staged:
            tc.stage_boundary()
        batches = [4, 4, 4, 4, 4, 2, 1]
        batch_start = 0
        yo = None
        for ht in range(KT):
            py = psum_y.tile([128, TT], fp32, tag="py")
            for it in range(MT):
                dsl = (0 if probe == 'pe1w'
                       else ht * ICP + it * 128)
                nc.tensor.matmul(
                    py[:],
                    dw_t[:, dsl:dsl + 128],
                    hglu[:, it * TT:(it + 1) * TT],
                    start=(it == 0), stop=(it == MT - 1))
            bi = ht - batch_start
            if bi == 0:
                nb = batches[0]
                yo = ypool.tile([128, nb * TT], bf16, tag="yo")
            # alternate PSUM->SBUF copies between DVE and ACT so the copy
            # stream keeps pace with the PE (one copy per ~650ns h-tile)
            if ht % 2 == 0:
                nc.vector.tensor_copy(yo[:, bi * TT:(bi + 1) * TT], py[:])
            else:
                nc.scalar.copy(yo[:, bi * TT:(bi + 1) * TT], py[:])
            if bi == nb - 1:
                h0 = ht - bi
                dst = y_d[h0 * 128:(h0 + nb) * 128, :].rearrange(
                    "(a p) t -> p a t", p=128)
                src_ap = yo[:].rearrange("p (a t) -> p a t", a=nb)
                if do_stores:
                    nc.scalar.dma_start(dst, src_ap)
                batches.pop(0)
                batch_start = ht + 1

    from contextlib import ExitStack
    with tile.TileContext(nc) as tc:
        with ExitStack() as ctx:
            pools = (
                ctx.enter_context(tc.tile_pool(name="w", bufs=1)),
                ctx.enter_context(tc.tile_pool(name="glu", bufs=3)),
                ctx.enter_context(tc.tile_pool(name="hglu", bufs=2)),
                ctx.enter_context(tc.tile_pool(name="psum", bufs=2,
                                               space="PSUM")),
                ctx.enter_context(tc.tile_pool(name="psum_y", bufs=4,
                                               space="PSUM")),
                ctx.enter_context(tc.tile_pool(name="yout", bufs=3)),
            )
            if unroll_bodies is not None:
                for bi in range(unroll_bodies):
                    body(ctx, tc, pools,
                         load_this_body=(probe not in ('pe', 'pe1w', 'gu', 'down') or bi == 0))
            elif loop_reps is None:
                body(ctx, tc, pools)
            else:
                assert loop_reps % unroll == 0
                with tc.For_i(0, loop_reps // unroll, 1,
                              hint_engines=(mybir.EngineType.PE,)):
                    for bi in range(unroll):
                        body(ctx, tc, pools,
                             load_this_body=(probe not in ('pe', 'pe1w', 'gu', 'down') or bi == 0))

    nc.compile()
    return nc


def prepare_in_maps(hidden_states, gate_w, gate_b, up_w, up_b, down_w,
                    active=None, tp=None):
    """Host-side shard + pad + pre-tile into the exact SBUF layouts.
    active/tp: token compaction — only hs[active] rows are shipped, padded
    to tp columns (defaults to all T tokens)."""
    hs = np.asarray(hidden_states, np.float32)
    if active is None:
        active = np.arange(T)
    if tp is None:
        tp = T
    hidT = np.zeros((HP, tp), np.float32)
    hidT[:H, :len(active)] = hs[active].T
    hid_tiled = np.ascontiguousarray(
        hidT.astype(BF16).reshape(KT, 128, tp).transpose(1, 0, 2)
    ).reshape(128, KT * tp)

    gw = np.asarray(gate_w, np.float32)
    uw = np.asarray(up_w, np.float32)
    dwf = np.asarray(down_w, np.float32)
    gbf = np.asarray(gate_b, np.float32).reshape(-1)
    ubf = np.asarray(up_b, np.float32).reshape(-1)

    def lhsT_tiles(Wp):  # [HP, 128] -> [128, KT*128]
        return np.ascontiguousarray(
            Wp.reshape(KT, 128, 128).transpose(1, 0, 2)).reshape(128, KT * 128)

    in_maps = []
    for c in range(NCORES):
        sl = slice(c * IC, (c + 1) * IC)
        Gp = np.zeros((HP, ICP), np.float32)
        Gp[:H, :IC] = gw[:, sl]
        Up = np.zeros((HP, ICP), np.float32)
        Up[:H, :IC] = uw[:, sl]
        Gp = Gp.astype(BF16)
        Up = Up.astype(BF16)
        blocks = []
        for m in range(MT):
            blocks.append(lhsT_tiles(Gp[:, m * 128:(m + 1) * 128]))
            blocks.append(lhsT_tiles(Up[:, m * 128:(m + 1) * 128]))
        gu = np.ascontiguousarray(np.concatenate(blocks, axis=1))

        Dp = np.zeros((ICP, HP), np.float32)
        # 1/ALPHA folded in: the device computes hglu = ALPHA*glu*(u+1)
        Dp[:IC, :H] = dwf[sl, :] * (1.0 / ALPHA)
        dw_tiled = np.ascontiguousarray(
            Dp.astype(BF16).reshape(MT, 128, KT, 128).transpose(1, 2, 0, 3)
        ).reshape(128, KT * MT * 128)

        gbp = np.zeros(ICP, np.float32)
        gbp[:IC] = gbf[sl]
        ubp = np.zeros(ICP, np.float32)
        ubp[:IC] = ubf[sl]

        in_maps.append({
            "hid": hid_tiled,
            "gu": gu,
            "dw": dw_tiled,
            "gb": np.ascontiguousarray(gbp.reshape(MT, 128).T),
            "ub": np.ascontiguousarray(ubp.reshape(MT, 128).T),
        })
    return in_maps


def kernel(hidden_states, routing_weights, final_hidden_states,
           gate_w, gate_b, up_w, up_b, down_w, down_b, expert_mask):
    from concourse.bass_utils import run_bass_kernel_spmd

    active, tok_w, tp = routing_compaction(expert_mask, routing_weights)
    out = np.array(np.asarray(final_hidden_states, np.float32), copy=True)
    if len(active) == 0:
        return out.astype(np.float32)

    if tp not in _cache:
        _cache[tp] = build_program(tp=tp)
    nc = _cache[tp]

    in_maps = prepare_in_maps(hidden_states, gate_w, gate_b, up_w, up_b,
                              down_w, active, tp)
    res = run_bass_kernel_spmd(nc, in_maps, list(range(NCORES)))

    ysum = np.zeros((HP, tp), np.float64)
    for c in range(NCORES):
        ysum += res.results[c]["y"].astype(np.float64)
    y = ysum[:H, :len(active)].T.astype(np.float32)     # [n_active, H]

    out[active] += ((y + np.asarray(down_b, np.float32).reshape(1, -1))
                    * tok_w[active, None])
    return out.astype(np.float32)



# revision 22
# speedup vs baseline: 1.1969x; 1.1969x over previous
"""GPT-OSS expert MLP (gate/up GEMM + clamped GLU + down GEMM + routing scale)
on 8 Trainium2 NeuronCores.

Sharding: tensor-parallel split of the intermediate dim I=2880 across 8 cores
(360 columns each, padded to 384 = 3*128). Each core computes
  gate/up = hidden @ W[:, slice] ; glu ; y_partial = glu_h @ down_w[slice, :]
and writes its full [H, T] partial (transposed layout). The host sums the 8
partials, applies down bias, routing weights, and the residual add.

All matmul operands are bf16: the quantized weights (values k/32, |k|<=4) are
exactly representable in bf16, so the only rounding is on hidden_states.
PSUM accumulation is fp32; partials are written out in bf16 and
summed on the host in fp64.
"""

import numpy as np
import ml_dtypes

BF16 = ml_dtypes.bfloat16

H = 2880          # hidden size
I = 2880          # intermediate size
T = 512           # tokens (full problem size)
NCORES = 8
IC = I // NCORES  # 360 intermediate cols per core
ICP = 384         # padded to 3 * 128
MT = ICP // 128   # 3 i-tiles per core
HP = 2944         # H padded to 23 * 128
KT = HP // 128    # 23 k-tiles over hidden dim
ALPHA = 1.702
LIMIT = 7.0
_cache = {}


def routing_compaction(expert_mask, routing_weights):
    """Tokens with sum_j mask[j,t]*rw[t,j] == 0 contribute exactly
    final_hidden_states[t] to the output, so the device only computes the
    active tokens. Returns (active_idx, tok_w, tp) with tp = active count
    padded up to a multiple of 64 (the compiled moving-dim size)."""
    mask = np.asarray(expert_mask, np.float32)
    rw = np.asarray(routing_weights, np.float32)
    tok_w = np.einsum("jt,tj->t", mask, rw)
    active = np.flatnonzero(tok_w)
    tp = max(64, -(-len(active) // 64) * 64)
    return active, tok_w, tp


def build_program(loop_reps=None, unroll_bodies=None, unroll=8, tp=448,
                  probe=None):
    """Build (and compile) the per-core Bass program. Identical on all cores;
    per-core data comes from in_maps.

    Structure (from HW microbenchmarks):
    - All streamed inputs are double-buffered (bufs=2), so body i+1's loads
      prefetch during body i with no WAR stall.
    - The down GEMM of body i is emitted interleaved between the six long
      gate/up accumulation chains of body i+1: short 3-matmul down chains
      with a PSUM->SBUF copy per chain throttle the PE to ~253 ns per
      448-col matmul when run back-to-back, but cost ~197 ns when spaced
      between 23-matmul chains (copy/drain interference amortizes).
    - Stores ride the ACT HWDGE ring so the SP ring only carries loads.
    loop_reps wraps `unroll` bodies in a For_i (timing only);
    unroll_bodies=N emits N bodies with no loop (sim only).
    """
    import concourse.bacc as bacc
    import concourse.mybir as mybir
    import concourse.tile as tile

    fp32 = mybir.dt.float32
    bf16 = mybir.dt.bfloat16

    nc = bacc.Bacc("TRN2", target_bir_lowering=False, debug=False,
                   num_devices=NCORES)

    TT = tp            # active-token count (moving dim)
    hid_d = nc.dram_tensor("hid", [128, KT * TT], bf16, kind="ExternalInput").ap()
    gu_d = nc.dram_tensor("gu", [128, 2 * MT * KT * 128], bf16,
                          kind="ExternalInput").ap()
    dw_d = nc.dram_tensor("dw", [128, KT * MT * 128], bf16,
                          kind="ExternalInput").ap()
    gb_d = nc.dram_tensor("gb", [128, MT], fp32, kind="ExternalInput").ap()
    ub_d = nc.dram_tensor("ub", [128, MT], fp32, kind="ExternalInput").ap()
    y_d = nc.dram_tensor("y", [HP, TT], bf16, kind="ExternalOutput").ap()

    # down h-tile groups: one store per group, interleaved 1:1 with the six
    # gate/up chains of the next body
    DGROUPS = [4, 4, 4, 4, 4, 3]
    do_loads = probe in (None, 'loads')
    do_compute = probe is None

    def emit_loads(pools):
        wpool = pools[0]
        hid_t = [None] * KT                    # kt -> (tile, col offset)
        gu_piece_sizes = {0: [6, 6, 6, 5], 1: [6, 6, 6, 5],
                          2: [12, 11], 3: [12, 11],
                          4: [12, 11], 5: [12, 11]}
        hid_piece_sizes = [2, 2, 2, 2, 3, 3, 3, 3, 3]
        gu_kt = {g: 0 for g in range(6)}
        hid_kt = [0]
        gu_map = {}

        def load_hid():
            ci = sum(1 for k in range(KT) if hid_t[k] is not None)
            nk = hid_piece_sizes.pop(0)
            kt0 = hid_kt[0]
            t = wpool.tile([128, nk * TT], bf16, tag=f"hid{ci}",
                           name=f"hid{ci}")
            if do_loads:
                nc.sync.dma_start(t[:], hid_d[:, kt0 * TT:(kt0 + nk) * TT])
            for j in range(nk):
                hid_t[kt0 + j] = (t, j)
            hid_kt[0] = kt0 + nk

        def load_gu_piece(grp, idx):
            nk = gu_piece_sizes[grp][idx]
            kt0 = gu_kt[grp]
            t = wpool.tile([128, nk * 128], bf16, tag=f"gu{grp}_{idx}",
                           name=f"gu{grp}_{idx}")
            if do_loads:
                nc.sync.dma_start(
                    t[:], gu_d[:, grp * KT * 128 + kt0 * 128:
                               grp * KT * 128 + (kt0 + nk) * 128])
            for j in range(nk):
                gu_map[(grp, kt0 + j)] = (t, j)
            gu_kt[grp] = kt0 + nk

        load_gu_piece(0, 0); load_hid(); load_gu_piece(0, 1); load_hid()
        load_gu_piece(0, 2); load_hid(); load_gu_piece(0, 3); load_hid()
        gb_t = wpool.tile([128, MT], fp32, tag="gb", name="gb")
        ub_t = wpool.tile([128, MT], fp32, tag="ub", name="ub")
        if do_loads:
            nc.sync.dma_start(gb_t[:], gb_d[:])
            nc.sync.dma_start(ub_t[:], ub_d[:])
        load_gu_piece(1, 0); load_hid(); load_gu_piece(1, 1); load_hid()
        load_gu_piece(1, 2); load_hid(); load_gu_piece(1, 3); load_hid()
        load_hid()
        for grp in (2, 3, 4, 5):
            load_gu_piece(grp, 0); load_gu_piece(grp, 1)
        dw_t = wpool.tile([128, KT * MT * 128], bf16, tag="dw", name="dw")
        if do_loads:
            nc.sync.dma_start(dw_t[:], dw_d[:])

        hglu = pools[2].tile([128, MT * TT], bf16, tag="hglu", name="hglu")
        return {"hid_t": hid_t, "gu_map": gu_map, "gb": gb_t, "ub": ub_t,
                "dw": dw_t, "hglu": hglu, "nch": 0, "yo": None}

    def emit_gu_chain(pools, h, c):
        """Chain c of 6: even = gate chain of m=c//2, odd = up chain + GLU."""
        wpool, glupool, hglupool, psum, psum_y, ypool = pools
        m = c // 2

        def rhs(kt):
            t, j = h["hid_t"][kt]
            return t[:, j * TT:(j + 1) * TT]

        def lhsT(grp, kt):
            t, j = h["gu_map"][(grp, kt)]
            return t[:, j * 128:(j + 1) * 128]

        if c % 2 == 0:
            pg = psum.tile([128, TT], fp32, tag="pg", name="pg")
            for kt in range(KT):
                nc.tensor.matmul(pg[:], lhsT(2 * m, kt), rhs(kt),
                                 start=(kt == 0), stop=(kt == KT - 1))
            h["pg"] = pg
        else:
            pu = psum.tile([128, TT], fp32, tag="pu", name="pu")
            for kt in range(KT):
                nc.tensor.matmul(pu[:], lhsT(2 * m + 1, kt), rhs(kt),
                                 start=(kt == 0), stop=(kt == KT - 1))
            pg = h["pg"]
            # gate: g = min(pg + gb, L); sg = silu(ALPHA*g) = ALPHA*glu
            tg = glupool.tile([128, TT], fp32, tag="tg", name="tg")
            nc.vector.tensor_scalar(tg[:], pg[:], h["gb"][:, m:m + 1], LIMIT,
                                    mybir.AluOpType.add, mybir.AluOpType.min)
            sg = glupool.tile([128, TT], fp32, tag="sg", name="sg")
            nc.scalar.activation(sg[:], tg[:],
                                 mybir.ActivationFunctionType.Silu,
                                 scale=ALPHA)
            # up: u = clip(pu + ub, -L, L); tu5 = u + 1
            tu = glupool.tile([128, TT], fp32, tag="tu", name="tu")
            nc.vector.tensor_scalar(tu[:], pu[:], h["ub"][:, m:m + 1], LIMIT,
                                    mybir.AluOpType.add, mybir.AluOpType.min)
            tu5 = glupool.tile([128, TT], fp32, tag="tu5", name="tu5")
            nc.vector.tensor_scalar(tu5[:], tu[:], -LIMIT, 1.0,
                                    mybir.AluOpType.max, mybir.AluOpType.add)
            # hglu = (ALPHA*glu) * (u+1); 1/ALPHA is folded into dw
            nc.vector.tensor_tensor(h["hglu"][:, m * TT:(m + 1) * TT],
                                    sg[:], tu5[:], mybir.AluOpType.mult)

    def emit_down_group(pools, h, g):
        """Down chains for DGROUPS[g] h-tiles + copies + one batched store."""
        wpool, glupool, hglupool, psum, psum_y, ypool = pools
        ht0 = sum(DGROUPS[:g])
        nb = DGROUPS[g]
        yo = ypool.tile([128, nb * TT], bf16, tag=f"yo{nb}", name=f"yo{nb}")
        for bi in range(nb):
            ht = ht0 + bi
            py = psum_y.tile([128, TT], fp32, tag="py", name="py")
            for it in range(MT):
                nc.tensor.matmul(
                    py[:],
                    h["dw"][:, ht * ICP + it * 128: ht * ICP + (it + 1) * 128],
                    h["hglu"][:, it * TT:(it + 1) * TT],
                    start=(it == 0), stop=(it == MT - 1))
            if h["nch"] % 2 == 0:
                nc.vector.tensor_copy(yo[:, bi * TT:(bi + 1) * TT], py[:])
            else:
                nc.scalar.copy(yo[:, bi * TT:(bi + 1) * TT], py[:])
            h["nch"] += 1
        dst = y_d[ht0 * 128:(ht0 + nb) * 128, :].rearrange(
            "(a p) t -> p a t", p=128)
        nc.scalar.dma_start(dst, yo[:].rearrange("p (a t) -> p a t", a=nb))

    def emit_bodies(pools, n):
        prev = None
        for _ in range(n):
            h = emit_loads(pools)
            if do_compute:
                for c in range(6):
                    emit_gu_chain(pools, h, c)
                    if prev is not None:
                        emit_down_group(pools, prev, c)
                prev = h
        if prev is not None and do_compute:
            for g in range(6):
                emit_down_group(pools, prev, g)

    from contextlib import ExitStack
    with tile.TileContext(nc) as tc:
        with ExitStack() as ctx:
            pools = (
                ctx.enter_context(tc.tile_pool(name="w", bufs=2)),
                ctx.enter_context(tc.tile_pool(name="glu", bufs=3)),
                ctx.enter_context(tc.tile_pool(name="hglu", bufs=2)),
                ctx.enter_context(tc.tile_pool(name="psum", bufs=2,
                                               space="PSUM")),
                ctx.enter_context(tc.tile_pool(name="psum_y", bufs=4,
                                               space="PSUM")),
                ctx.enter_context(tc.tile_pool(name="yout", bufs=3)),
            )
            if unroll_bodies is not None:
                emit_bodies(pools, unroll_bodies)
            elif loop_reps is None:
                emit_bodies(pools, 1)
            else:
                assert loop_reps % unroll == 0
                with tc.For_i(0, loop_reps // unroll, 1,
                              hint_engines=(mybir.EngineType.PE,)):
                    emit_bodies(pools, unroll)

    nc.compile()
    return nc


def prepare_in_maps(hidden_states, gate_w, gate_b, up_w, up_b, down_w,
                    active=None, tp=None):
    """Host-side shard + pad + pre-tile into the exact SBUF layouts.
    active/tp: token compaction — only hs[active] rows are shipped, padded
    to tp columns (defaults to all T tokens)."""
    hs = np.asarray(hidden_states, np.float32)
    if active is None:
        active = np.arange(T)
    if tp is None:
        tp = T
    hidT = np.zeros((HP, tp), np.float32)
    hidT[:H, :len(active)] = hs[active].T
    hid_tiled = np.ascontiguousarray(
        hidT.astype(BF16).reshape(KT, 128, tp).transpose(1, 0, 2)
    ).reshape(128, KT * tp)

    gw = np.asarray(gate_w, np.float32)
    uw = np.asarray(up_w, np.float32)
    dwf = np.asarray(down_w, np.float32)
    gbf = np.asarray(gate_b, np.float32).reshape(-1)
    ubf = np.asarray(up_b, np.float32).reshape(-1)

    def lhsT_tiles(Wp):  # [HP, 128] -> [128, KT*128]
        return np.ascontiguousarray(
            Wp.reshape(KT, 128, 128).transpose(1, 0, 2)).reshape(128, KT * 128)

    in_maps = []
    for c in range(NCORES):
        sl = slice(c * IC, (c + 1) * IC)
        Gp = np.zeros((HP, ICP), np.float32)
        Gp[:H, :IC] = gw[:, sl]
        Up = np.zeros((HP, ICP), np.float32)
        Up[:H, :IC] = uw[:, sl]
        Gp = Gp.astype(BF16)
        Up = Up.astype(BF16)
        blocks = []
        for m in range(MT):
            blocks.append(lhsT_tiles(Gp[:, m * 128:(m + 1) * 128]))
            blocks.append(lhsT_tiles(Up[:, m * 128:(m + 1) * 128]))
        gu = np.ascontiguousarray(np.concatenate(blocks, axis=1))

        Dp = np.zeros((ICP, HP), np.float32)
        # 1/ALPHA folded in: the device computes hglu = ALPHA*glu*(u+1)
        Dp[:IC, :H] = dwf[sl, :] * (1.0 / ALPHA)
        dw_tiled = np.ascontiguousarray(
            Dp.astype(BF16).reshape(MT, 128, KT, 128).transpose(1, 2, 0, 3)
        ).reshape(128, KT * MT * 128)

        gbp = np.zeros(ICP, np.float32)
        gbp[:IC] = gbf[sl]
        ubp = np.zeros(ICP, np.float32)
        ubp[:IC] = ubf[sl]

        in_maps.append({
            "hid": hid_tiled,
            "gu": gu,
            "dw": dw_tiled,
            "gb": np.ascontiguousarray(gbp.reshape(MT, 128).T),
            "ub": np.ascontiguousarray(ubp.reshape(MT, 128).T),
        })
    return in_maps


def kernel(hidden_states, routing_weights, final_hidden_states,
           gate_w, gate_b, up_w, up_b, down_w, down_b, expert_mask):
    from concourse.bass_utils import run_bass_kernel_spmd

    active, tok_w, tp = routing_compaction(expert_mask, routing_weights)
    out = np.array(np.asarray(final_hidden_states, np.float32), copy=True)
    if len(active) == 0:
        return out.astype(np.float32)

    if tp not in _cache:
        _cache[tp] = build_program(tp=tp)
    nc = _cache[tp]

    in_maps = prepare_in_maps(hidden_states, gate_w, gate_b, up_w, up_b,
                              down_w, active, tp)
    res = run_bass_kernel_spmd(nc, in_maps, list(range(NCORES)))

    ysum = np.zeros((HP, tp), np.float64)
    for c in range(NCORES):
        ysum += res.results[c]["y"].astype(np.float64)
    y = ysum[:H, :len(active)].T.astype(np.float32)     # [n_active, H]

    out[active] += ((y + np.asarray(down_b, np.float32).reshape(1, -1))
                    * tok_w[active, None])
    return out.astype(np.float32)

